# revision 34
# baseline (speedup 1.0000x reference)
"""Trainium2 Bass kernel for nn_AttentionDecoder (GRU decoder + dot attention).

Strategy (8 NeuronCores, data-parallel over batch, no collectives):
  - batch 64 -> 8 per core
  - Phase A (parallel): giT = W_ih @ embed^T for all timesteps (bf16 matmul)
  - Phase B (serial, 128 steps): GRU recurrence in transposed layout
    (gate-dim on partitions, batch on free dim). W_hh stationary bf16.
    Gates are split into 4 h-chunks whose DVE/ACT chains are emitted as
    software-pipelined segments interleaved with the matmul wavefront, so
    neither the PE nor the in-order DVE/ACT FIFOs stall.
  - Phase C (parallel): attention per batch element via bf16 PE matmuls,
    free-dim softmax, PE transposes to assemble the output.

All matmuls use bf16 operands with f32 PSUM accumulation; gate arithmetic is
f32 (h is rounded to bf16 once per step). Host side does only sharding,
layout transposes, dtype casts, and the embedding gather.
"""

import numpy as np
import ml_dtypes

NB, S, H, E = 8, 128, 512, 512
G = 3 * H            # 1536
BT = NB * S          # 1024
NCORES = 8

_cache = {}


def _build():
    import concourse.bass as bass
    import concourse.bacc as bacc
    import concourse.mybir as mybir
    from concourse import tile
    from contextlib import ExitStack

    f32 = mybir.dt.float32
    bf16 = mybir.dt.bfloat16
    AF = mybir.ActivationFunctionType
    ALU = mybir.AluOpType
    PSUM = bass.MemorySpace.PSUM

    nc = bacc.Bacc(
        "TRN2",
        target_bir_lowering=False,
        debug=False,
        enable_asserts=False,
        num_devices=NCORES,
    )

    embedT_d = nc.dram_tensor("embedT", [E, BT], bf16, kind="ExternalInput")
    wih_d = nc.dram_tensor("W_ihT", [E, G], bf16, kind="ExternalInput")
    whh_d = nc.dram_tensor("W_hhT", [H, G], bf16, kind="ExternalInput")
    biascol_d = nc.dram_tensor("bias_col", [128, 12], f32, kind="ExternalInput")
    bhhn_d = nc.dram_tensor("bhh_n", [128, 4, NB], bf16, kind="ExternalInput")
    h0T_d = nc.dram_tensor("h0T", [H, NB], bf16, kind="ExternalInput")
    enc_d = nc.dram_tensor("enc", [NB, S, H], bf16, kind="ExternalInput")
    encT_d = nc.dram_tensor("encT", [NB, H, S], bf16, kind="ExternalInput")
    iden_d = nc.dram_tensor("iden", [128, 128], bf16, kind="ExternalInput")
    out_d = nc.dram_tensor("out", [NB, S, 2 * H], f32, kind="ExternalOutput")

    with tile.TileContext(nc) as tc, ExitStack() as ctx:
        cp = ctx.enter_context(tc.tile_pool(name="const", bufs=1))
        giT = cp.tile([128, 12, BT], f32)            # [p, g*4+hm, t*8+b]
        # h storage doubles as the recurrent state: column t holds h_{t-1}
        HallT = cp.tile([128, 4, NB, S + 1], bf16)   # [p, hm, b, t]
        whh = cp.tile([128, 4, G], bf16)
        wih = cp.tile([128, 4, G], bf16)
        embT = cp.tile([128, 4, BT], bf16)
        biascol = cp.tile([128, 12], f32)
        bhhn = cp.tile([128, 4, NB], bf16)
        iden = cp.tile([128, 128], bf16)

        nc.sync.dma_start(whh[:], whh_d.ap().rearrange("(k p) g -> p k g", p=128))
        nc.sync.dma_start(wih[:], wih_d.ap().rearrange("(k p) g -> p k g", p=128))
        nc.sync.dma_start(embT[:], embedT_d.ap().rearrange("(k p) n -> p k n", p=128))
        nc.sync.dma_start(biascol[:], biascol_d.ap())
        nc.sync.dma_start(bhhn[:], bhhn_d.ap())
        h0t = cp.tile([128, 4, NB], bf16)
        nc.sync.dma_start(h0t[:], h0T_d.ap().rearrange("(k p) b -> p k b", p=128))
        nc.vector.tensor_copy(HallT[:, :, :, 0], h0t[:])
        nc.sync.dma_start(iden[:], iden_d.ap())

        # ---- Phase A: giT[:, m, :] = (W_ih chunk) @ embedT + (b_ih [+ b_hh]) ----
        with tc.tile_pool(name="psA", bufs=4, space=PSUM) as psA:
            for m in range(12):
                for n in range(2):
                    psa = psA.tile([128, 512], f32, tag="psa")
                    for k in range(4):
                        nc.tensor.matmul(
                            psa[:],
                            wih[:, k, 128 * m : 128 * (m + 1)],
                            embT[:, k, 512 * n : 512 * (n + 1)],
                            start=(k == 0),
                            stop=(k == 3),
                        )
                    dst = giT[:, m, 512 * n : 512 * (n + 1)]
                    if (2 * m + n) % 2 == 0:
                        nc.vector.tensor_scalar_add(dst, psa[:], biascol[:, m : m + 1])
                    else:
                        nc.scalar.activation(
                            dst, psa[:], AF.Identity, bias=biascol[:, m : m + 1]
                        )

        # ---- Phase B: GRU recurrence, 128 serial steps ----
        # Weight m-index for gate g of h-chunk hm is m = 4*g + hm.
        # Two gate chunks (hm 0:2 and 2:4); for each, a 6-op DVE chain with
        # two ACT hops, software-pipelined so the in-order queues stay fed.
        # h lives in HallT column t (bf16); matmul rhs reads it strided.
        # PSUM groups are strictly sequential within the (single) bank:
        # for each m, its 4-5 matmuls (identity(b_hh) for n-gates, then
        # k0..k3) are consecutive. m-order is chunk-major so the first gate
        # chunk's inputs complete early; gate chains are software-pipelined
        # across the step boundary, tails on the GPSIMD engine.
        with (
            tc.tile_pool(name="psB", bufs=2, space=PSUM) as psB,
            tc.tile_pool(name="gp", bufs=3) as gp,
        ):
            state = {}   # (t, kind, c) -> tile, for cross-step pipelining

            def seg_a(st, psg, gig, c):  # c = 0 -> hm 0:2, c = 1 -> hm 2:4
                hs = slice(2 * c, 2 * c + 2)
                trz = gp.tile([128, 2, 2, NB], f32, tag=f"trz{c}", name=f"trz{c}")
                nc.vector.tensor_add(trz[:], psg[:, 0:2, hs, :], gig[:, 0:2, hs, :])
                rz = gp.tile([128, 2, 2, NB], f32, tag=f"rz{c}", name=f"rz{c}")
                nc.scalar.activation(rz[:], trz[:], AF.Sigmoid)
                state[(st, "rz", c)] = rz

            def seg_b(st, psg, gig, c):
                hs = slice(2 * c, 2 * c + 2)
                rz = state[(st, "rz", c)]
                tn2 = gp.tile([128, 2, NB], f32, tag=f"tn2{c}", name=f"tn2{c}")
                nc.vector.tensor_mul(tn2[:], rz[:, 0, :, :], psg[:, 2, hs, :])
                tn3 = gp.tile([128, 2, NB], f32, tag=f"tn3{c}", name=f"tn3{c}")
                nc.vector.tensor_add(tn3[:], tn2[:], gig[:, 2, hs, :])
                nn = gp.tile([128, 2, NB], f32, tag=f"nn{c}", name=f"nn{c}")
                nc.scalar.activation(nn[:], tn3[:], AF.Tanh)
                state[(st, "nn", c)] = nn

            def seg_c(st, c):
                # tail runs on the otherwise-idle GPSIMD engine (SBUF-only)
                hs = slice(2 * c, 2 * c + 2)
                rz = state.pop((st, "rz", c))
                nn = state.pop((st, "nn", c))
                th = gp.tile([128, 2, NB], f32, tag=f"th{c}", name=f"th{c}")
                nc.gpsimd.tensor_sub(th[:], HallT[:, hs, :, st], nn[:])
                th2 = gp.tile([128, 2, NB], f32, tag=f"th2{c}", name=f"th2{c}")
                nc.gpsimd.tensor_mul(th2[:], rz[:, 1, :, :], th[:])
                nc.gpsimd.tensor_add(HallT[:, hs, :, st + 1], nn[:], th2[:])

            def m_group(psb, h_src, m, c):
                if m >= 8:   # open the n-gate group with the b_hh identity MM
                    nc.tensor.matmul(
                        psb[:, m, :], iden[:], bhhn[:, m - 8, :],
                        start=True, stop=False,
                    )
                for k in range(4):
                    nc.tensor.matmul(
                        psb[:, m, :],
                        whh[:, k, 128 * m : 128 * (m + 1)],
                        h_src[:, k, :],
                        start=(k == 0 and m < 8),
                        stop=(k == 3),
                    )

            prev = None  # (psg, gig) of step t-1 for cross-step tail segs
            for t in range(S):
                h_src = HallT[:, :, :, t]            # [128, 4, NB]
                psb = psB.tile([128, 12, NB], f32, tag="psb", name="psb")
                psg = psb[:].rearrange("p (g m) b -> p g m b", g=3)
                gig = giT[:, :, 8 * t : 8 * (t + 1)].rearrange(
                    "p (g m) b -> p g m b", g=3
                )
                if prev is not None:
                    seg_b(t - 1, *prev, 1)
                    seg_c(t - 1, 1)
                m_group(psb, h_src, 0, 0)
                m_group(psb, h_src, 1, 0)
                m_group(psb, h_src, 4, 0)
                m_group(psb, h_src, 5, 0)
                m_group(psb, h_src, 8, 0)
                m_group(psb, h_src, 9, 0)
                seg_a(t, psg, gig, 0)
                m_group(psb, h_src, 2, 1)
                m_group(psb, h_src, 3, 1)
                seg_b(t, psg, gig, 0)
                m_group(psb, h_src, 6, 1)
                seg_c(t, 0)
                m_group(psb, h_src, 7, 1)
                m_group(psb, h_src, 10, 1)
                m_group(psb, h_src, 11, 1)
                seg_a(t, psg, gig, 1)
                prev = (psg, gig)
            seg_b(S - 1, *prev, 1)
            seg_c(S - 1, 1)

        # ---- Phase C: attention + output assembly, per batch element ----
        with (
            tc.tile_pool(name="pc", bufs=2) as pc,
            tc.tile_pool(name="psC", bufs=2, space=PSUM) as psC,
            tc.tile_pool(name="psX", bufs=2, space=PSUM) as psX,
        ):
            for b in range(NB):
                encb = pc.tile([128, H], bf16, tag="encb")
                nc.sync.dma_start(encb[:], enc_d.ap()[b])
                enctb = pc.tile([128, 4, S], bf16, tag="enctb")
                nc.sync.dma_start(
                    enctb[:], encT_d.ap()[b].rearrange("(k p) s -> p k s", p=128)
                )
                ps_sc = psC.tile([128, 128], f32, tag="c128")
                for k in range(4):
                    nc.tensor.matmul(
                        ps_sc[:],
                        HallT[:, k, b, 1 : S + 1],
                        enctb[:, k, :],
                        start=(k == 0),
                        stop=(k == 3),
                    )
                mxn = pc.tile([128, 1], f32, tag="mxn")
                nc.vector.tensor_reduce(
                    mxn[:], ps_sc[:], op=ALU.max,
                    axis=mybir.AxisListType.X, negate=True,
                )
                probs = pc.tile([128, 128], bf16, tag="probs")
                sm = pc.tile([128, 1], f32, tag="sm")
                nc.scalar.activation(
                    probs[:], ps_sc[:], AF.Exp, bias=mxn[:], accum_out=sm[:]
                )
                rs = pc.tile([128, 1], f32, tag="rs")
                nc.vector.reciprocal(rs[:], sm[:])
                ps_pt = psC.tile([128, 128], bf16, tag="c128b", bufs=4)
                nc.tensor.transpose(ps_pt[:], probs[:], iden[:])
                probsT = pc.tile([128, 128], bf16, tag="probsT")
                nc.vector.tensor_copy(probsT[:], ps_pt[:])
                ps_cx = psX.tile([128, 512], f32, tag="ctx")
                nc.tensor.matmul(ps_cx[:], probsT[:], encb[:], start=True, stop=True)
                y = pc.tile([128, 2 * H], f32, tag="y")
                for k in range(4):
                    ps_h = psC.tile([128, 128], bf16, tag="c128b", bufs=4)
                    nc.tensor.transpose(ps_h[:], HallT[:, k, b, 1 : S + 1], iden[:])
                    nc.vector.tensor_copy(y[:, 128 * k : 128 * (k + 1)], ps_h[:])
                nc.vector.tensor_scalar_mul(y[:, H:], ps_cx[:], rs[:])
                nc.sync.dma_start(out_d.ap()[b], y[:])

    nc.compile()
    return nc


def _get_nc():
    if "nc" not in _cache:
        _cache["nc"] = _build()
    return _cache["nc"]


def prepare_in_maps(
    decoder_input,
    encoder_hidden,
    encoder_output,
    emb_table,
    W_ih,
    W_hh,
    b_ih,
    b_hh,
    epoch=0,
    **_unused,
):
    dec = np.asarray(decoder_input)
    enc_h = np.asarray(encoder_hidden, np.float32)[0]      # [64, 512]
    enc_o = np.asarray(encoder_output, np.float32)         # [64, 128, 512]
    emb = np.asarray(emb_table, np.float32)
    W_ih = np.asarray(W_ih, np.float32)
    W_hh = np.asarray(W_hh, np.float32)
    b_ih = np.asarray(b_ih, np.float32)
    b_hh = np.asarray(b_hh, np.float32)

    embed = emb[dec]                                       # [64, 128, 512] gather

    WihT_bf = np.ascontiguousarray(W_ih.T).astype(ml_dtypes.bfloat16)
    WhhT_bf = np.ascontiguousarray(W_hh.T).astype(ml_dtypes.bfloat16)
    # bias_col[:, m] = b_ih chunk m, plus b_hh chunk for r/z gates (m < 8)
    bias_col = np.zeros((128, 12), np.float32)
    for m in range(12):
        bias_col[:, m] = b_ih[128 * m : 128 * (m + 1)]
        if m < 8:
            bias_col[:, m] += b_hh[128 * m : 128 * (m + 1)]
    # bhh_n[p, k, b] = b_hh[1024 + 128k + p]
    bhh_n = np.ascontiguousarray(
        np.repeat(b_hh[1024:].reshape(4, 128).T[:, :, None], NB, axis=2)
    ).astype(ml_dtypes.bfloat16)
    iden = np.eye(128, dtype=ml_dtypes.bfloat16)

    in_maps = []
    for c in range(NCORES):
        bs = slice(c * NB, (c + 1) * NB)
        embedT = np.ascontiguousarray(
            embed[bs].transpose(2, 1, 0).reshape(E, BT)
        ).astype(ml_dtypes.bfloat16)                       # [E, t*8+b]
        enc_c = enc_o[bs]
        in_maps.append(
            {
                "embedT": embedT,
                "W_ihT": WihT_bf,
                "W_hhT": WhhT_bf,
                "bias_col": bias_col,
                "bhh_n": bhh_n,
                "h0T": np.ascontiguousarray(enc_h[bs].T).astype(ml_dtypes.bfloat16),
                "enc": np.ascontiguousarray(enc_c).astype(ml_dtypes.bfloat16),
                "encT": np.ascontiguousarray(
                    enc_c.transpose(0, 2, 1)
                ).astype(ml_dtypes.bfloat16),
                "iden": iden,
            }
        )
    return in_maps


def assemble(results):
    out = np.empty((NCORES * NB, S, 2 * H), np.float32)
    for c in range(NCORES):
        out[c * NB : (c + 1) * NB] = results[c]["out"]
    return out


def kernel(**inputs):
    from concourse.bass_utils import run_bass_kernel_spmd

    in_maps = prepare_in_maps(**inputs)
    nc = _get_nc()
    _cache["in_maps"] = in_maps
    res = run_bass_kernel_spmd(nc, in_maps, core_ids=list(range(NCORES)))
    return assemble(res.results)
